# revision 20
# baseline (speedup 1.0000x reference)
"""Trainium2 Bass kernel for nn_Connection_75411035783724 (Mamba2 block + MLP head).

Sharding: tensor-parallel over the 32 Mamba2 heads across 8 cores (4 heads each).
Each core computes the in_proj column-slice it needs (its x-channels + B + dt),
the depthwise causal conv (as 4 accumulating diagonal matmuls on the PE),
and a chunked-SSD scan (chunk 256): per-chunk states via PE matmuls over
transposed activations, inter-chunk recurrence via a single tensor_tensor_scan.
Only the 32 frame-start tokens are ever projected to the output (the reference
discards all other rows), so the gated RMSNorm + out_proj + MLP run on 32 rows
only (launch 2, column-sharded MLP2).

Key numeric choices: bf16 for matmul operands and big intermediates, fp32 PSUM
accumulation and fp32 for the dt/decay pipeline.
"""
import os
import sys
import numpy as np
import ml_dtypes

sys.path.insert(0, "/opt/trn_rl_repo")

import concourse.bass as bass
import concourse.tile as tile
import concourse.mybir as mybir
from concourse import bacc
from concourse import bass_utils

F32 = mybir.dt.float32
BF16 = mybir.dt.bfloat16
AF = mybir.ActivationFunctionType
OP = mybir.AluOpType
BF = ml_dtypes.bfloat16

# Model dims
D_MODEL = 1024
HIDDEN = 4096
D_STATE = 128       # n
D_CONV = 4
D_INNER = 2048
HEADDIM = 64        # p
NHEADS = 32
CONV_DIM = D_INNER + 2 * D_STATE            # 2304
D_IN_PROJ = 2 * D_INNER + 2 * D_STATE + NHEADS  # 4384
L = 8192            # tokens
NPOS = 32           # output positions (first token of each frame)
POS_STRIDE = 256
NCORES = 8
HPC = 4             # heads per core
Q = 256             # chunk length
NCHUNK = L // Q     # 32
KT = D_MODEL // 128  # 8 K-tiles
NG = 16             # token groups of 512
GSZ = 512
# in_proj col slice per core: [x 256 | B 128 | dt 4]
NCOL = 256 + 128 + 4  # 388
MT_SPEC = [(0, 128), (128, 128), (256, 128), (384, 4)]  # (col0, width)


def _bf(x):
    return np.ascontiguousarray(np.asarray(x, dtype=np.float32)).astype(BF)


def _f32(x):
    return np.ascontiguousarray(np.asarray(x, dtype=np.float32))


# ----------------------------------------------------------------------------
# Launch 1 program: in_proj + conv + scan -> gated y at the 32 positions
# ----------------------------------------------------------------------------
_L1 = None


def build_l1():
    global _L1
    if _L1 is not None:
        return _L1
    nc = bacc.Bacc("TRN2", target_bir_lowering=False, debug=False,
                   num_devices=NCORES)
    dram = {}

    def din(name, shape, dt):
        dram[name] = nc.dram_tensor(name, shape, dt, kind="ExternalInput").ap()
        return dram[name]

    xT = din("xT", (D_MODEL, L), BF16)
    xTpos = din("xTpos", (D_MODEL, NPOS), BF16)
    xTwin = din("xTwin", (D_MODEL, NPOS * D_CONV), BF16)
    w_in = din("w_in", (KT, 128, NCOL), BF16)
    w_c = din("w_c", (KT, 128, 128), BF16)
    w_z = din("w_z", (KT, 128, 256), BF16)
    diag_w = din("diag_w", (3, D_CONV, 128, 128), BF16)
    cw_c = din("cw_c", (128, D_CONV), F32)
    conv_b = din("conv_b", (128, 3), F32)
    conv_b_c = din("conv_b_c", (128, 1), F32)
    dtb_hc = din("dtb_hc", (128, 1), F32)
    A_hc = din("A_hc", (128, 1), F32)
    D_hc = din("D_hc", (128, 1), F32)
    mask_c0 = din("mask_c0", (128, 1), F32)
    y32g_out = nc.dram_tensor("y32g", (128, 2, NPOS), F32,
                              kind="ExternalOutput").ap()

    with tile.TileContext(nc) as tc:
        import contextlib
        with contextlib.ExitStack() as ctx:
            sb = ctx.enter_context(tc.tile_pool(name="sb", bufs=1))
            ring = ctx.enter_context(tc.tile_pool(name="ring", bufs=1))
            dsc = ctx.enter_context(tc.tile_pool(name="dsc", bufs=1, space="DRAM"))

            # ---- resident weights/constants
            w_in_sb = sb.tile([128, KT, NCOL], BF16)
            nc.sync.dma_start(out=w_in_sb, in_=w_in.rearrange("k p c -> p k c"))
            w_c_sb = sb.tile([128, KT, 128], BF16)
            nc.sync.dma_start(out=w_c_sb, in_=w_c.rearrange("k p c -> p k c"))
            w_z_sb = sb.tile([128, KT, 256], BF16)
            nc.sync.dma_start(out=w_z_sb, in_=w_z.rearrange("k p c -> p k c"))
            diag_sb = sb.tile([128, 3, D_CONV, 128], BF16)
            nc.sync.dma_start(out=diag_sb, in_=diag_w.rearrange("c j a b -> a c j b"))
            cw_sb = sb.tile([128, D_CONV], F32)
            nc.sync.dma_start(out=cw_sb, in_=cw_c)
            cb_sb = sb.tile([128, 3], F32)
            nc.sync.dma_start(out=cb_sb, in_=conv_b)
            cbc_sb = sb.tile([128, 1], F32)
            nc.sync.dma_start(out=cbc_sb, in_=conv_b_c)
            dtb_sb = sb.tile([128, 1], F32)
            nc.sync.dma_start(out=dtb_sb, in_=dtb_hc)
            A_sb = sb.tile([128, 1], F32)
            nc.sync.dma_start(out=A_sb, in_=A_hc)
            D_sb = sb.tile([128, 1], F32)
            nc.sync.dma_start(out=D_sb, in_=D_hc)
            m0_sb = sb.tile([128, 1], F32)
            nc.sync.dma_start(out=m0_sb, in_=mask_c0)
            xtp_sb = sb.tile([128, KT, NPOS], BF16)
            nc.sync.dma_start(out=xtp_sb, in_=xTpos.rearrange("(k p) t -> p k t", p=128))
            xtw_sb = sb.tile([128, KT, NPOS * D_CONV], BF16)
            nc.sync.dma_start(out=xtw_sb, in_=xTwin.rearrange("(k p) t -> p k t", p=128))

            # ---- persistent big buffers (S_all/Sg recycle xbcc/XT slots via tags)
            xbcc = sb.tile([128, 3, L], BF16, tag="big1")  # conv+silu [x0|x1|B]
            dt_d = dsc.tile([HPC, L], F32)             # dt_raw parked in DRAM
            x32 = sb.tile([128, 2, NPOS], F32)
            B32 = sb.tile([128, NPOS], F32)
            XT = sb.tile([128, L // 128, 256], BF16, tag="big2")  # transposed x*w
            BT = sb.tile([128, L // 128, 128], BF16)   # transposed B
            ones_t = sb.tile([128, Q], F32)
            nc.vector.memset(ones_t, 1.0)

            # ================= phase A: in_proj + conv + silu =================
            with contextlib.ExitStack() as pctx:
                psA = pctx.enter_context(tc.tile_pool(name="psA", bufs=1, space="PSUM"))
                prev_xbc = None
                for g in range(NG):
                    sl = slice(g * GSZ, (g + 1) * GSZ)
                    xt_g = ring.tile([128, KT, GSZ], BF16, tag="xt", bufs=2)
                    nc.sync.dma_start(
                        out=xt_g,
                        in_=xT.rearrange("(k p) t -> p k t", p=128)[:, :, sl])
                    # in_proj matmuls
                    ps = []
                    for mt, (c0, cw) in enumerate(MT_SPEC):
                        p = psA.tile([cw, GSZ], F32, tag=f"pin{mt}")
                        for k in range(KT):
                            nc.tensor.matmul(p, w_in_sb[:, k, c0:c0 + cw],
                                             xt_g[:, k, :],
                                             start=(k == 0), stop=(k == KT - 1))
                        ps.append(p)
                    # evict to xbc ring (3 halo cols at the front)
                    xbc_g = ring.tile([128, 3, GSZ + 3], BF16, tag="xbc", bufs=3)
                    if prev_xbc is None:
                        nc.vector.memset(xbc_g[:, :, 0:3], 0.0)
                    else:
                        nc.vector.tensor_copy(out=xbc_g[:, :, 0:3],
                                              in_=prev_xbc[:, :, GSZ:GSZ + 3])
                    for cht in range(3):
                        if cht != 2:
                            nc.vector.tensor_copy(out=xbc_g[:, cht, 3:], in_=ps[cht])
                        else:
                            nc.scalar.copy(out=xbc_g[:, cht, 3:], in_=ps[cht])
                    dt_ev = ring.tile([HPC, GSZ], F32, tag="dtev", bufs=2)
                    nc.vector.tensor_copy(out=dt_ev, in_=ps[3])
                    nc.sync.dma_start(out=dt_d[:, sl], in_=dt_ev)
                    prev_xbc = xbc_g
                    # conv: 4 accumulating diag matmuls per channel tile
                    for cht in range(3):
                        pc = psA.tile([128, GSZ], F32, tag="pcv", bufs=2)
                        for j in range(D_CONV):
                            nc.tensor.matmul(pc, diag_sb[:, cht, j, :],
                                             xbc_g[:, cht, j:j + GSZ],
                                             start=(j == 0), stop=(j == D_CONV - 1))
                        # silu(conv + bias) eviction
                        nc.scalar.activation(out=xbcc[:, cht, sl], in_=pc,
                                             func=AF.Silu,
                                             bias=cb_sb[:, cht:cht + 1], scale=1.0)
                    # extract positions 2g, 2g+1 (cols 0 and 256 of this group)
                    for cht in range(2):
                        s_ap = bass.AP(
                            tensor=xbcc.tensor, offset=xbcc.offset + cht * L + g * GSZ,
                            ap=[list(xbcc.ap[0]), [POS_STRIDE, 2]])
                        nc.vector.tensor_copy(out=x32[:, cht, 2 * g:2 * g + 2], in_=s_ap)
                    s_ap = bass.AP(
                        tensor=xbcc.tensor, offset=xbcc.offset + 2 * L + g * GSZ,
                        ap=[list(xbcc.ap[0]), [POS_STRIDE, 2]])
                    nc.vector.tensor_copy(out=B32[:, 2 * g:2 * g + 2], in_=s_ap)
                    # B transpose can start now (no decay-weight dependency)
                    nc.sync.dma_start_transpose(
                        out=BT[:, 4 * g:4 * g + 4, :], in_=xbcc[:, 2, sl])

            # ================= dt pipeline =================
            dt32 = sb.tile([128, Q], F32)
            nc.sync.dma_start(out=dt32, in_=dt_d.rearrange("h (c q) -> (h c) q", q=Q))
            # softplus(v) = v - ln(sigmoid(v)), v = dt_raw + dt_bias
            nc.vector.tensor_scalar(dt32, dt32, dtb_sb[:, 0:1], None, OP.add)
            sgm = sb.tile([128, Q], F32)
            nc.scalar.activation(out=sgm, in_=dt32, func=AF.Sigmoid)
            nc.scalar.activation(out=sgm, in_=sgm, func=AF.Ln)
            nc.vector.tensor_sub(dt32, dt32, sgm)
            a_t = sb.tile([128, Q], F32)
            nc.vector.tensor_scalar_mul(a_t, dt32, A_sb[:, 0:1])
            s_t = sb.tile([128, Q], F32)
            nc.vector.tensor_tensor_scan(out=s_t, data0=ones_t, data1=a_t,
                                         initial=0.0, op0=OP.mult, op1=OP.add)
            stot = sb.tile([128, 1], F32)
            nc.vector.tensor_copy(out=stot, in_=s_t[:, Q - 1:Q])
            sms = sb.tile([128, Q], F32)
            nc.vector.tensor_scalar(sms, s_t, stot[:, 0:1], None, OP.subtract)
            w_hc = sb.tile([128, Q], F32)
            # w = exp(stot - s) = exp(-(s - stot))
            nc.scalar.activation(out=w_hc, in_=sms, func=AF.Exp, scale=-1.0)
            nc.vector.tensor_mul(w_hc, w_hc, dt32)
            w16 = sb.tile([128, Q], BF16)
            nc.vector.tensor_copy(out=w16, in_=w_hc)
            lam = sb.tile([128, 1], F32)
            nc.scalar.activation(out=lam, in_=stot, func=AF.Exp)
            nc.vector.tensor_mul(lam, lam, m0_sb)
            lam16 = sb.tile([128, 1], BF16)
            nc.vector.tensor_copy(out=lam16, in_=lam)
            dA_hc = sb.tile([128, 1], F32)
            nc.scalar.activation(out=dA_hc, in_=a_t[:, 0:1], func=AF.Exp)

            # ---- DRAM roundtrips: w16 -> per-group bcast; lam -> lam_flat
            w_d = dsc.tile([128, Q], BF16)
            nc.sync.dma_start(out=w_d, in_=w16)
            lam_d = dsc.tile([128, 1], BF16)
            nc.sync.dma_start(out=lam_d, in_=lam16)
            lam_src = bass.AP(tensor=lam_d.tensor, offset=lam_d.offset,
                              ap=[[0, 1], [NCHUNK, HPC], [0, HEADDIM], [1, NCHUNK]])
            lam_row_d = dsc.tile([1, HPC * HEADDIM * NCHUNK], BF16)
            nc.sync.dma_start(out=lam_row_d.rearrange("o (h p c) -> o h p c",
                                                      h=HPC, p=HEADDIM),
                              in_=lam_src)
            lam_flat = sb.tile([128, HPC * HEADDIM * NCHUNK], BF16)
            nc.sync.dma_start(out=lam_flat,
                              in_=bass.AP(tensor=lam_row_d.tensor,
                                          offset=lam_row_d.offset,
                                          ap=[[0, 128], [1, HPC * HEADDIM * NCHUNK]]))

            # ================= phase B: scale + transpose x =================
            for g in range(NG):
                sl = slice(g * GSZ, (g + 1) * GSZ)
                wbc_g = ring.tile([128, 2, GSZ], BF16, tag="wbc", bufs=2)
                for t in range(2):
                    src = bass.AP(tensor=w_d.tensor,
                                  offset=w_d.offset + (2 * t) * L + g * GSZ,
                                  ap=[[L, 2], [0, 64], [1, GSZ]])
                    nc.sync.dma_start(out=wbc_g[:, t, :], in_=src)
                xs_g = ring.tile([128, 2, GSZ], BF16, tag="xs", bufs=2)
                for t in range(2):
                    nc.vector.tensor_mul(xs_g[:, t, :], xbcc[:, t, sl], wbc_g[:, t, :])
                    nc.sync.dma_start_transpose(
                        out=XT[:, 4 * g:4 * g + 4, t * 128:(t + 1) * 128],
                        in_=xs_g[:, t, :])

            # ================= phase C: scan =================
            with contextlib.ExitStack() as pctx:
                psC = pctx.enter_context(tc.tile_pool(name="psC", bufs=1, space="PSUM"))
                # recycle the xbcc slot for the chunk-local states
                S_all = sb.tile([128, HPC * HEADDIM * NCHUNK], F32, tag="big1")
                for c in range(NCHUNK):
                    # all 4 heads' columns are contiguous in XT -> one matmul
                    pc = psC.tile([128, HPC * HEADDIM], F32, tag="psc", bufs=4)
                    for k2 in range(2):
                        T = 2 * c + k2
                        nc.tensor.matmul(pc, BT[:, T, :], XT[:, T, :],
                                         start=(k2 == 0), stop=(k2 == 1))
                    dst = bass.AP(tensor=S_all.tensor, offset=S_all.offset + c,
                                  ap=[list(S_all.ap[0]),
                                      [HEADDIM * NCHUNK, HPC], [NCHUNK, HEADDIM]])
                    eng = nc.vector if c % 2 == 0 else nc.scalar
                    if c % 2 == 0:
                        nc.vector.tensor_copy(out=dst, in_=pc.rearrange(
                            "n (h p) -> n h p", h=HPC))
                    else:
                        nc.scalar.copy(out=dst, in_=pc.rearrange(
                            "n (h p) -> n h p", h=HPC))
                # inter-chunk recurrence: state = lam*state + S_local along c
                # (Sg recycles the XT slot -- all scan matmuls have read XT by now)
                Sg = sb.tile([128, HPC * HEADDIM * NCHUNK], BF16, tag="big2")
                nc.vector.tensor_tensor_scan(out=Sg, data0=lam_flat, data1=S_all,
                                             initial=0.0, op0=OP.mult, op1=OP.add)

                # ================= y32 + gate =================
                C32 = sb.tile([128, NPOS], F32)
                pcw = psC.tile([128, NPOS * D_CONV], F32, tag="pcw")
                for k in range(KT):
                    nc.tensor.matmul(pcw, w_c_sb[:, k, :], xtw_sb[:, k, :],
                                     start=(k == 0), stop=(k == KT - 1))
                tmpc = sb.tile([128, NPOS], F32)
                for j in range(D_CONV):
                    src = bass.AP(tensor=pcw.tensor, offset=pcw.offset + j,
                                  ap=[list(pcw.ap[0]), [D_CONV, NPOS]])
                    if j == 0:
                        nc.vector.tensor_scalar_mul(tmpc, src, cw_sb[:, 0:1])
                    else:
                        nc.vector.scalar_tensor_tensor(
                            out=tmpc, in0=src, scalar=cw_sb[:, j:j + 1], in1=tmpc,
                            op0=OP.mult, op1=OP.add)
                nc.scalar.activation(out=C32, in_=tmpc, func=AF.Silu,
                                     bias=cbc_sb[:, 0:1], scale=1.0)
                C32b = sb.tile([128, NPOS], BF16)
                nc.vector.tensor_copy(out=C32b, in_=C32)

                # BC row = sum_n B32*C32 -> [1, NPOS]
                bc_t = sb.tile([128, NPOS], F32)
                nc.vector.tensor_mul(bc_t, B32, C32)
                ones1 = sb.tile([128, 1], F32)
                nc.vector.memset(ones1, 1.0)
                pbc = psC.tile([1, NPOS], F32, tag="pbc")
                nc.tensor.matmul(pbc, ones1, bc_t, start=True, stop=True)
                bc_row = sb.tile([1, NPOS], F32)
                nc.vector.tensor_copy(out=bc_row, in_=pbc)
                bc_d = dsc.tile([1, NPOS], F32)
                nc.sync.dma_start(out=bc_d, in_=bc_row)
                bc_hc = sb.tile([128, 1], F32)
                nc.sync.dma_start(out=bc_hc,
                                  in_=bass.AP(tensor=bc_d.tensor, offset=bc_d.offset,
                                              ap=[[0, HPC], [1, NPOS], [0, 1]]))
                # f_hc = dt*.BC* + D  (per (h,c)=(h,pos) partition)
                f_hc = sb.tile([128, 1], F32)
                nc.vector.scalar_tensor_tensor(out=f_hc, in0=dt32[:, 0:1],
                                               scalar=bc_hc[:, 0:1], in1=D_sb,
                                               op0=OP.mult, op1=OP.add)
                # broadcast dA_hc, f_hc -> [128,(t,pos)] via DRAM
                small_d = dsc.tile([128, 2], F32)
                nc.sync.dma_start(out=small_d[:, 0:1], in_=dA_hc)
                nc.sync.dma_start(out=small_d[:, 1:2], in_=f_hc)

                def bcast_hp(col):
                    t_sb = sb.tile([128, 2, NPOS], F32, tag=f"bch{col}")
                    for t in range(2):
                        for hh in range(2):
                            src = bass.AP(
                                tensor=small_d.tensor,
                                offset=small_d.offset + col + (2 * t + hh) * 2 * NCHUNK,
                                ap=[[0, 64], [2, NPOS]])
                            nc.sync.dma_start(out=t_sb[64 * hh:64 * (hh + 1), t, :],
                                              in_=src)
                    return t_sb

                dA_bc = bcast_hp(0)
                f_bc = bcast_hp(1)

                # per-position projections: y32s[(hh,p), t, pos] = C32 . Sg[:,(h,p,pos-1)]
                py = psC.tile([128, 2, NPOS], F32, tag="py")
                nc.vector.memset(py[:, :, 0:1], 0.0)
                first = True
                for pos in range(1, NPOS):
                    for t in range(2):
                        lhs = bass.AP(
                            tensor=Sg.tensor,
                            offset=Sg.offset + (2 * t) * HEADDIM * NCHUNK + (pos - 1),
                            ap=[list(Sg.ap[0]), [HEADDIM * NCHUNK, 2], [NCHUNK, 64]])
                        nc.tensor.matmul(py[:, t, pos:pos + 1], lhs,
                                         C32b[:, pos:pos + 1],
                                         start=first, stop=(pos == NPOS - 1 and t == 1),
                                         skip_group_check=True)
                        first = False
                y32 = sb.tile([128, 2, NPOS], F32)
                nc.vector.tensor_mul(y32, py, dA_bc)
                tloc = sb.tile([128, 2, NPOS], F32)
                nc.vector.tensor_mul(tloc, x32, f_bc)
                nc.vector.tensor_add(y32, y32, tloc)

                # z gate
                pz = psC.tile([128, 2, NPOS], F32, tag="pz")
                for t in range(2):
                    for k in range(KT):
                        nc.tensor.matmul(pz[:, t, :],
                                         w_z_sb[:, k, t * 128:(t + 1) * 128],
                                         xtp_sb[:, k, :],
                                         start=(k == 0), stop=(k == KT - 1))
                zs = sb.tile([128, 2, NPOS], F32)
                nc.scalar.activation(out=zs, in_=pz, func=AF.Silu)
                nc.vector.tensor_mul(y32, y32, zs)
                nc.sync.dma_start(out=y32g_out, in_=y32)

    nc.compile()
    _L1 = nc
    return nc


# ----------------------------------------------------------------------------
# Launch 2 program: gated RMSNorm + out_proj + MLP on the 32 rows
# ----------------------------------------------------------------------------
_L2 = None


def build_l2():
    global _L2
    if _L2 is not None:
        return _L2
    nc = bacc.Bacc("TRN2", target_bir_lowering=False, debug=False,
                   num_devices=NCORES)

    y32g = nc.dram_tensor("y32g_full", (128, 16, NPOS), F32, kind="ExternalInput").ap()
    norm_w = nc.dram_tensor("norm_w", (128, 16), F32, kind="ExternalInput").ap()
    w_out = nc.dram_tensor("w_outT", (16, 128, D_MODEL), BF16, kind="ExternalInput").ap()
    w1 = nc.dram_tensor("w1T", (KT, 128, HIDDEN), BF16, kind="ExternalInput").ap()
    b1 = nc.dram_tensor("b1", (128, HIDDEN // 128), F32, kind="ExternalInput").ap()
    w2 = nc.dram_tensor("w2T", (32, 128, 512), BF16, kind="ExternalInput").ap()
    b2 = nc.dram_tensor("b2", (128, 4), F32, kind="ExternalInput").ap()
    out32 = nc.dram_tensor("out32", (128, 4, NPOS), F32, kind="ExternalOutput").ap()

    with tile.TileContext(nc) as tc:
        import contextlib
        with contextlib.ExitStack() as ctx:
            sb = ctx.enter_context(tc.tile_pool(name="sb", bufs=1))
            psp = ctx.enter_context(tc.tile_pool(name="ps", bufs=1, space="PSUM"))
            dsc = ctx.enter_context(tc.tile_pool(name="dsc", bufs=1, space="DRAM"))

            y_sb = sb.tile([128, 16, NPOS], F32)
            nc.sync.dma_start(out=y_sb, in_=y32g)
            nw_sb = sb.tile([128, 16], F32)
            nc.sync.dma_start(out=nw_sb, in_=norm_w)
            wo_sb = sb.tile([128, 16, D_MODEL], BF16)
            nc.sync.dma_start(out=wo_sb, in_=w_out.rearrange("k p c -> p k c"))
            w1_sb = sb.tile([128, KT, HIDDEN], BF16)
            nc.sync.dma_start(out=w1_sb, in_=w1.rearrange("k p c -> p k c"))
            b1_sb = sb.tile([128, HIDDEN // 128], F32)
            nc.sync.dma_start(out=b1_sb, in_=b1)
            w2_sb = sb.tile([128, 32, 512], BF16)
            nc.sync.dma_start(out=w2_sb, in_=w2.rearrange("k p c -> p k c"))
            b2_sb = sb.tile([128, 4], F32)
            nc.sync.dma_start(out=b2_sb, in_=b2)

            # sum of squares over channels (partition x 16 ktiles)
            sq = sb.tile([128, 16, NPOS], F32)
            nc.vector.tensor_mul(sq, y_sb, y_sb)
            ones1 = sb.tile([128, 1], F32)
            nc.vector.memset(ones1, 1.0)
            pss = psp.tile([1, NPOS], F32, tag="pss")
            for k in range(16):
                nc.tensor.matmul(pss, ones1, sq[:, k, :],
                                 start=(k == 0), stop=(k == 15))
            # r = 1/sqrt(mean + eps)
            eps_t = sb.tile([1, 1], F32)
            nc.vector.memset(eps_t, 1e-5)
            rs = sb.tile([1, NPOS], F32)
            nc.scalar.activation(out=rs, in_=pss, func=AF.Sqrt,
                                 bias=eps_t[:, 0:1], scale=1.0 / D_INNER)
            nc.vector.reciprocal(rs, rs)
            r_d = dsc.tile([1, NPOS], F32)
            nc.sync.dma_start(out=r_d, in_=rs)
            r_bc = sb.tile([128, NPOS], F32)
            nc.sync.dma_start(out=r_bc,
                              in_=bass.AP(tensor=r_d.tensor, offset=r_d.offset,
                                          ap=[[0, 128], [1, NPOS]]))
            yn = sb.tile([128, 16, NPOS], BF16)
            for k in range(16):
                nc.vector.scalar_tensor_tensor(out=yn[:, k, :], in0=y_sb[:, k, :],
                                               scalar=nw_sb[:, k:k + 1], in1=r_bc,
                                               op0=OP.mult, op1=OP.mult)
            # h = w_outT.T @ yn   [1024, 32]
            h_sb = sb.tile([128, 8, NPOS], BF16)
            for mt in range(8):
                ph = psp.tile([128, NPOS], F32, tag="ph", bufs=2)
                for k in range(16):
                    nc.tensor.matmul(ph, wo_sb[:, k, mt * 128:(mt + 1) * 128],
                                     yn[:, k, :], start=(k == 0), stop=(k == 15))
                nc.vector.tensor_copy(out=h_sb[:, mt, :], in_=ph)
            # g = gelu(w1T.T @ h + b1)  [4096, 32]
            g_sb = sb.tile([128, 32, NPOS], BF16)
            for mt in range(32):
                pg = psp.tile([128, NPOS], F32, tag="pg", bufs=2)
                for k in range(KT):
                    nc.tensor.matmul(pg, w1_sb[:, k, mt * 128:(mt + 1) * 128],
                                     h_sb[:, k, :], start=(k == 0), stop=(k == KT - 1))
                nc.scalar.activation(out=g_sb[:, mt, :], in_=pg, func=AF.Gelu,
                                     bias=b1_sb[:, mt:mt + 1], scale=1.0)
            # out = w2T.T @ g + b2   [512, 32] per core
            for mt in range(4):
                po = psp.tile([128, NPOS], F32, tag="po", bufs=2)
                for k in range(32):
                    nc.tensor.matmul(po, w2_sb[:, k, mt * 128:(mt + 1) * 128],
                                     g_sb[:, k, :], start=(k == 0), stop=(k == 31))
                ot = sb.tile([128, NPOS], F32, tag="ot", bufs=2)
                nc.vector.tensor_scalar(ot, po, b2_sb[:, mt:mt + 1], None, OP.add)
                nc.sync.dma_start(out=out32[:, mt, :], in_=ot)

    nc.compile()
    _L2 = nc
    return nc


# ----------------------------------------------------------------------------
# Host-side prep + glue
# ----------------------------------------------------------------------------

def _prep_l1_maps(inputs):
    x = _f32(inputs["x"]).reshape(L, D_MODEL)
    xT = np.ascontiguousarray(x.T)                       # [1024, 8192]
    xT_b = _bf(xT)
    pos = np.arange(NPOS) * POS_STRIDE
    xTpos = _bf(xT[:, pos])
    # window tokens (pos, d): t*-3+d, zero-padded below 0
    win_idx = (pos[:, None] + np.arange(D_CONV)[None, :] - (D_CONV - 1)).reshape(-1)
    xTwin = np.zeros((D_MODEL, NPOS * D_CONV), np.float32)
    valid = win_idx >= 0
    xTwin[:, valid] = xT[:, win_idx[valid]]
    xTwin = _bf(xTwin)

    w_all = _f32(inputs["in_proj_w"])                    # [4384, 1024]
    conv_w = _f32(inputs["conv_w"])                      # [2304, 4]
    conv_b = _f32(inputs["conv_b"])                      # [2304]
    dt_bias = _f32(inputs["dt_bias"])                    # [32]
    A = -np.exp(_f32(inputs["A_log"]))                   # [32]
    Dp = _f32(inputs["D"])                               # [32]

    w_cT = _bf(w_all[D_INNER + D_INNER + D_STATE:
                     D_INNER + D_INNER + 2 * D_STATE].T.reshape(KT, 128, 128))
    cw_c = _f32(conv_w[D_INNER + D_STATE:])              # [128, 4] C channels
    conv_b_c = _f32(conv_b[D_INNER + D_STATE:]).reshape(128, 1)

    maps = []
    for k in range(NCORES):
        xs = 256 * k
        cols = np.concatenate([
            np.arange(D_INNER + xs, D_INNER + xs + 256),          # x slice
            np.arange(2 * D_INNER, 2 * D_INNER + D_STATE),        # B
            np.arange(D_IN_PROJ - NHEADS + HPC * k,
                      D_IN_PROJ - NHEADS + HPC * k + HPC),        # dt
        ])
        w_in = _bf(w_all[cols].T.reshape(KT, 128, NCOL))
        w_z = _bf(w_all[xs:xs + 256].T.reshape(KT, 128, 256))
        # conv channels for this core: x slice (256) + B (128)
        ch_x = np.arange(xs, xs + 256)
        ch_B = np.arange(D_INNER, D_INNER + D_STATE)
        dw = np.zeros((3, D_CONV, 128, 128), np.float32)
        cb = np.zeros((128, 3), np.float32)
        for cht, chs in enumerate([ch_x[:128], ch_x[128:], ch_B]):
            for j in range(D_CONV):
                dw[cht, j] = np.diag(conv_w[chs, j])
            cb[:, cht] = conv_b[chs]
        heads = np.arange(HPC * k, HPC * k + HPC)
        dtb_hc = np.repeat(dt_bias[heads], NCHUNK).reshape(128, 1).astype(np.float32)
        A_hc = np.repeat(A[heads], NCHUNK).reshape(128, 1).astype(np.float32)
        D_hc = np.repeat(Dp[heads], NCHUNK).reshape(128, 1).astype(np.float32)
        mask = np.ones((128, 1), np.float32)
        mask[::NCHUNK] = 0.0
        maps.append({
            "xT": xT_b, "xTpos": xTpos, "xTwin": xTwin,
            "w_in": w_in, "w_c": w_cT, "w_z": w_z,
            "diag_w": _bf(dw), "cw_c": cw_c, "conv_b": cb,
            "conv_b_c": conv_b_c, "dtb_hc": dtb_hc, "A_hc": A_hc,
            "D_hc": D_hc, "mask_c0": mask,
        })
    return maps


def _prep_l2_maps(inputs, y32g_full):
    # ch = kt*128 + p -> norm_w_sb[p, kt] = norm_w[kt*128+p]
    nw = _f32(inputs["norm_w"]).reshape(16, 128).transpose(1, 0).copy()
    w_out = _f32(inputs["mamba_out_w"])                  # [1024, 2048]
    w_outT = _bf(w_out.T.reshape(16, 128, D_MODEL))
    w1 = _f32(inputs["mlp_w1"])                          # [4096, 1024]
    w1T = _bf(w1.T.reshape(KT, 128, HIDDEN))
    b1 = _f32(inputs["mlp_b1"]).reshape(32, 128).transpose(1, 0).copy()
    w2 = _f32(inputs["mlp_w2"])                          # [4096, 4096]
    maps = []
    for k in range(NCORES):
        cols = slice(512 * k, 512 * k + 512)
        w2T = _bf(w2[cols].T.reshape(32, 128, 512))
        b2 = _f32(inputs["mlp_b2"])[cols].reshape(4, 128).transpose(1, 0).copy()
        maps.append({
            "y32g_full": y32g_full, "norm_w": nw, "w_outT": w_outT,
            "w1T": w1T, "b1": b1, "w2T": w2T, "b2": b2,
        })
    return maps


LAST_RESULTS = []


def kernel(**inputs) -> np.ndarray:
    trace = os.environ.get("KERNEL_TRACE", "0") == "1"
    LAST_RESULTS.clear()
    nc1 = build_l1()
    maps1 = _prep_l1_maps(inputs)
    res1 = bass_utils.run_bass_kernel_spmd(nc1, maps1, core_ids=list(range(NCORES)),
                                           trace=trace)
    LAST_RESULTS.append(res1)
    # assemble y32g_full [128, 16, 32]: ch = 256*k + t*128 + p -> kt = 2k+t
    y32g_full = np.zeros((128, 16, NPOS), np.float32)
    for k in range(NCORES):
        y = res1.results[k]["y32g"]                      # [128, 2, 32]
        y32g_full[:, 2 * k:2 * k + 2, :] = y
    nc2 = build_l2()
    maps2 = _prep_l2_maps(inputs, y32g_full)
    res2 = bass_utils.run_bass_kernel_spmd(nc2, maps2, core_ids=list(range(NCORES)),
                                           trace=trace)
    LAST_RESULTS.append(res2)
    out = np.zeros((NPOS, HIDDEN), np.float32)
    for k in range(NCORES):
        o = res2.results[k]["out32"]                     # [128, 4, 32]
        # out[pos, 512k + mt*128 + p] = o[p, mt, pos]
        out[:, 512 * k:512 * (k + 1)] = o.transpose(2, 1, 0).reshape(NPOS, 512)
    return out.astype(np.float32)


# revision 26
# speedup vs baseline: 1.1657x; 1.1657x over previous
"""Trainium2 Bass kernel for nn_Connection_75411035783724 (Mamba2 block + MLP head).

Sharding: tensor-parallel over the 32 Mamba2 heads across 8 cores (4 heads each).
Each core computes the in_proj column-slice it needs (its x-channels + B + dt),
the depthwise causal conv (as 4 accumulating diagonal matmuls on the PE),
and a chunked-SSD scan (chunk 256): per-chunk states via PE matmuls over
transposed activations, inter-chunk recurrence via a single tensor_tensor_scan.
Only the 32 frame-start tokens are ever projected to the output (the reference
discards all other rows), so the gated RMSNorm + out_proj + MLP run on 32 rows
only (launch 2, column-sharded MLP2).

Key numeric choices: bf16 for matmul operands and big intermediates, fp32 PSUM
accumulation and fp32 for the dt/decay pipeline.
"""
import os
import sys
import numpy as np
import ml_dtypes

sys.path.insert(0, "/opt/trn_rl_repo")

import concourse.bass as bass
import concourse.tile as tile
import concourse.mybir as mybir
from concourse import bacc
from concourse import bass_utils

F32 = mybir.dt.float32
BF16 = mybir.dt.bfloat16
AF = mybir.ActivationFunctionType
OP = mybir.AluOpType
BF = ml_dtypes.bfloat16

# Model dims
D_MODEL = 1024
HIDDEN = 4096
D_STATE = 128       # n
D_CONV = 4
D_INNER = 2048
HEADDIM = 64        # p
NHEADS = 32
CONV_DIM = D_INNER + 2 * D_STATE            # 2304
D_IN_PROJ = 2 * D_INNER + 2 * D_STATE + NHEADS  # 4384
L = 8192            # tokens
NPOS = 32           # output positions (first token of each frame)
POS_STRIDE = 256
NCORES = 8
HPC = 4             # heads per core
Q = 256             # chunk length
NCHUNK = L // Q     # 32
KT = D_MODEL // 128  # 8 K-tiles
NG = 16             # token groups of 512
GSZ = 512
# in_proj col slice per core: [x 256 | B 128 | dt 4]
NCOL = 256 + 128 + 4  # 388
MT_SPEC = [(0, 128), (128, 128), (256, 128), (384, 4)]  # (col0, width)


def _bf(x):
    return np.ascontiguousarray(np.asarray(x, dtype=np.float32)).astype(BF)


def _f32(x):
    return np.ascontiguousarray(np.asarray(x, dtype=np.float32))


# ----------------------------------------------------------------------------
# Launch 1 program: in_proj + conv + scan -> gated y at the 32 positions
# ----------------------------------------------------------------------------
_L1 = None


def build_l1():
    global _L1
    if _L1 is not None:
        return _L1
    nc = bacc.Bacc("TRN2", target_bir_lowering=False, debug=False,
                   num_devices=NCORES)
    dram = {}

    def din(name, shape, dt):
        dram[name] = nc.dram_tensor(name, shape, dt, kind="ExternalInput").ap()
        return dram[name]

    xT = din("xT", (D_MODEL, L), BF16)
    xTpos = din("xTpos", (D_MODEL, NPOS), BF16)
    xTwin = din("xTwin", (D_MODEL, NPOS * D_CONV), BF16)
    w_in = din("w_in", (KT, 128, NCOL), BF16)
    w_c = din("w_c", (KT, 128, 128), BF16)
    w_z = din("w_z", (KT, 128, 256), BF16)
    diag_w = din("diag_w", (3, D_CONV, 128, 128), BF16)
    cw_c = din("cw_c", (128, D_CONV), F32)
    conv_b = din("conv_b", (128, 3), F32)
    conv_b_c = din("conv_b_c", (128, 1), F32)
    dtb_hc = din("dtb_hc", (128, 1), F32)
    A_hc = din("A_hc", (128, 1), F32)
    D_hc = din("D_hc", (128, 1), F32)
    mask_c0 = din("mask_c0", (128, 1), F32)
    y32g_out = nc.dram_tensor("y32g", (128, 2, NPOS), F32,
                              kind="ExternalOutput").ap()

    with tile.TileContext(nc) as tc:
        import contextlib
        with contextlib.ExitStack() as ctx:
            sb = ctx.enter_context(tc.tile_pool(name="sb", bufs=1))
            ring = ctx.enter_context(tc.tile_pool(name="ring", bufs=1))
            dsc = ctx.enter_context(tc.tile_pool(name="dsc", bufs=1, space="DRAM"))

            # ---- resident weights/constants
            w_in_sb = sb.tile([128, KT, NCOL], BF16)
            nc.sync.dma_start(out=w_in_sb, in_=w_in.rearrange("k p c -> p k c"))
            w_c_sb = sb.tile([128, KT, 128], BF16)
            nc.sync.dma_start(out=w_c_sb, in_=w_c.rearrange("k p c -> p k c"))
            w_z_sb = sb.tile([128, KT, 256], BF16)
            nc.sync.dma_start(out=w_z_sb, in_=w_z.rearrange("k p c -> p k c"))
            diag_sb = sb.tile([128, 3, D_CONV, 128], BF16)
            nc.sync.dma_start(out=diag_sb, in_=diag_w.rearrange("c j a b -> a c j b"))
            cw_sb = sb.tile([128, D_CONV], F32)
            nc.sync.dma_start(out=cw_sb, in_=cw_c)
            cb_sb = sb.tile([128, 3], F32)
            nc.sync.dma_start(out=cb_sb, in_=conv_b)
            cbc_sb = sb.tile([128, 1], F32)
            nc.sync.dma_start(out=cbc_sb, in_=conv_b_c)
            dtb_sb = sb.tile([128, 1], F32)
            nc.sync.dma_start(out=dtb_sb, in_=dtb_hc)
            A_sb = sb.tile([128, 1], F32)
            nc.sync.dma_start(out=A_sb, in_=A_hc)
            D_sb = sb.tile([128, 1], F32)
            nc.sync.dma_start(out=D_sb, in_=D_hc)
            m0_sb = sb.tile([128, 1], F32)
            nc.sync.dma_start(out=m0_sb, in_=mask_c0)
            xtp_sb = sb.tile([128, KT, NPOS], BF16)
            nc.sync.dma_start(out=xtp_sb, in_=xTpos.rearrange("(k p) t -> p k t", p=128))
            xtw_sb = sb.tile([128, KT, NPOS * D_CONV], BF16)
            nc.sync.dma_start(out=xtw_sb, in_=xTwin.rearrange("(k p) t -> p k t", p=128))

            # ---- persistent big buffers (S_all/Sg recycle xbcc/XT slots via tags)
            xbcc = sb.tile([128, 3, L], BF16, tag="big1")  # conv+silu [x0|x1|B]
            dt_d = dsc.tile([HPC, L], F32)             # dt_raw parked in DRAM
            x32 = sb.tile([128, 2, NPOS], F32)
            B32 = sb.tile([128, NPOS], F32)
            XT = sb.tile([128, L // 128, 256], BF16, tag="big2")  # transposed x*w
            BT = sb.tile([128, L // 128, 128], BF16)   # transposed B
            ones_t = sb.tile([128, Q], F32)
            nc.vector.memset(ones_t, 1.0)

            # ================= phase A: in_proj + conv + silu =================
            with contextlib.ExitStack() as pctx:
                psA = pctx.enter_context(tc.tile_pool(name="psA", bufs=1, space="PSUM"))
                prev_xbc = None
                xt_h = None
                for g in range(NG):
                    sl = slice(g * GSZ, (g + 1) * GSZ)
                    if g % 2 == 0:
                        # batched load: 2KB-contiguous runs per (p,k)
                        xt_h = ring.tile([128, KT, 2 * GSZ], BF16, tag="xt", bufs=2)
                        hsl = slice(g * GSZ, (g + 2) * GSZ)
                        nc.sync.dma_start(
                            out=xt_h,
                            in_=xT.rearrange("(k p) t -> p k t", p=128)[:, :, hsl])
                    xt_g = xt_h[:, :, (g % 2) * GSZ:(g % 2 + 1) * GSZ]
                    # in_proj matmuls
                    ps = []
                    for mt, (c0, cw) in enumerate(MT_SPEC):
                        p = psA.tile([cw, GSZ], F32, tag=f"pin{mt}")
                        for k in range(KT):
                            nc.tensor.matmul(p, w_in_sb[:, k, c0:c0 + cw],
                                             xt_g[:, k, :],
                                             start=(k == 0), stop=(k == KT - 1))
                        ps.append(p)
                    # evict to xbc ring (3 halo cols at the front)
                    xbc_g = ring.tile([128, 3, GSZ + 3], BF16, tag="xbc", bufs=3)
                    if prev_xbc is None:
                        nc.vector.memset(xbc_g[:, :, 0:3], 0.0)
                    else:
                        nc.vector.tensor_copy(out=xbc_g[:, :, 0:3],
                                              in_=prev_xbc[:, :, GSZ:GSZ + 3])
                    for cht in range(3):
                        if cht != 2:
                            nc.vector.tensor_copy(out=xbc_g[:, cht, 3:], in_=ps[cht])
                        else:
                            nc.scalar.copy(out=xbc_g[:, cht, 3:], in_=ps[cht])
                    dt_ev = ring.tile([HPC, GSZ], F32, tag="dtev", bufs=2)
                    nc.vector.tensor_copy(out=dt_ev, in_=ps[3])
                    nc.gpsimd.dma_start(out=dt_d[:, sl], in_=dt_ev)
                    prev_xbc = xbc_g
                    # conv: 4 accumulating diag matmuls per channel tile
                    for cht in range(3):
                        pc = psA.tile([128, GSZ], F32, tag="pcv", bufs=2)
                        for j in range(D_CONV):
                            nc.tensor.matmul(pc, diag_sb[:, cht, j, :],
                                             xbc_g[:, cht, j:j + GSZ],
                                             start=(j == 0), stop=(j == D_CONV - 1))
                        # silu(conv + bias) eviction
                        nc.scalar.activation(out=xbcc[:, cht, sl], in_=pc,
                                             func=AF.Silu,
                                             bias=cb_sb[:, cht:cht + 1], scale=1.0)
                    # B transpose, batched per 2048 tokens
                    if g % 4 == 3:
                        qsl = slice((g - 3) * GSZ, (g + 1) * GSZ)
                        nc.scalar.dma_start_transpose(
                            out=BT[:, 4 * (g - 3):4 * (g + 1), :],
                            in_=xbcc[:, 2, qsl])
                # position extracts (cols 0/256 of each group), batched
                for cht in range(2):
                    s_ap = bass.AP(tensor=xbcc.tensor, offset=xbcc.offset + cht * L,
                                   ap=[list(xbcc.ap[0]), [POS_STRIDE, NPOS]])
                    nc.vector.tensor_copy(out=x32[:, cht, :], in_=s_ap)
                s_ap = bass.AP(tensor=xbcc.tensor, offset=xbcc.offset + 2 * L,
                               ap=[list(xbcc.ap[0]), [POS_STRIDE, NPOS]])
                nc.vector.tensor_copy(out=B32, in_=s_ap)

            # ================= dt pipeline =================
            dt32 = sb.tile([128, Q], F32)
            nc.sync.dma_start(out=dt32, in_=dt_d.rearrange("h (c q) -> (h c) q", q=Q))
            # softplus(v) = v - ln(sigmoid(v)), v = dt_raw + dt_bias
            nc.vector.tensor_scalar(dt32, dt32, dtb_sb[:, 0:1], None, OP.add)
            sgm = sb.tile([128, Q], F32)
            nc.scalar.activation(out=sgm, in_=dt32, func=AF.Sigmoid)
            nc.scalar.activation(out=sgm, in_=sgm, func=AF.Ln)
            nc.vector.tensor_sub(dt32, dt32, sgm)
            a_t = sb.tile([128, Q], F32)
            nc.vector.tensor_scalar_mul(a_t, dt32, A_sb[:, 0:1])
            s_t = sb.tile([128, Q], F32)
            nc.vector.tensor_tensor_scan(out=s_t, data0=ones_t, data1=a_t,
                                         initial=0.0, op0=OP.mult, op1=OP.add)
            stot = sb.tile([128, 1], F32)
            nc.vector.tensor_copy(out=stot, in_=s_t[:, Q - 1:Q])
            sms = sb.tile([128, Q], F32)
            nc.vector.tensor_scalar(sms, s_t, stot[:, 0:1], None, OP.subtract)
            w_hc = sb.tile([128, Q], F32)
            # w = exp(stot - s) = exp(-(s - stot))
            nc.scalar.activation(out=w_hc, in_=sms, func=AF.Exp, scale=-1.0)
            nc.vector.tensor_mul(w_hc, w_hc, dt32)
            w16 = sb.tile([128, Q], BF16)
            nc.vector.tensor_copy(out=w16, in_=w_hc)
            lam = sb.tile([128, 1], F32)
            nc.scalar.activation(out=lam, in_=stot, func=AF.Exp)
            nc.vector.tensor_mul(lam, lam, m0_sb)
            lam16 = sb.tile([128, 1], BF16)
            nc.vector.tensor_copy(out=lam16, in_=lam)
            dA_hc = sb.tile([128, 1], F32)
            nc.scalar.activation(out=dA_hc, in_=a_t[:, 0:1], func=AF.Exp)

            # ---- DRAM roundtrips: w16 -> per-group bcast; lam -> lam_flat
            w_d = dsc.tile([128, Q], BF16)
            nc.sync.dma_start(out=w_d, in_=w16)
            lam_d = dsc.tile([128, 1], BF16)
            nc.sync.dma_start(out=lam_d, in_=lam16)
            lam_src = bass.AP(tensor=lam_d.tensor, offset=lam_d.offset,
                              ap=[[0, 1], [NCHUNK, HPC], [0, HEADDIM], [1, NCHUNK]])
            lam_row_d = dsc.tile([1, HPC * HEADDIM * NCHUNK], BF16)
            nc.sync.dma_start(out=lam_row_d.rearrange("o (h p c) -> o h p c",
                                                      h=HPC, p=HEADDIM),
                              in_=lam_src)
            lam_flat = sb.tile([128, HPC * HEADDIM * NCHUNK], BF16)
            nc.sync.dma_start(out=lam_flat,
                              in_=bass.AP(tensor=lam_row_d.tensor,
                                          offset=lam_row_d.offset,
                                          ap=[[0, 128], [1, HPC * HEADDIM * NCHUNK]]))

            # ================= phase B: scale + transpose x (1K-token blocks) =
            QSZ = 2 * GSZ  # 1024 tokens
            for q in range(8):
                sl = slice(q * QSZ, (q + 1) * QSZ)
                wbc_q = ring.tile([128, 2, QSZ], BF16, tag="wbc", bufs=2)
                for t in range(2):
                    src = bass.AP(tensor=w_d.tensor,
                                  offset=w_d.offset + (2 * t) * L + q * QSZ,
                                  ap=[[L, 2], [0, 64], [1, QSZ]])
                    nc.gpsimd.dma_start(out=wbc_q[:, t, :], in_=src)
                xs_q = ring.tile([128, 2, QSZ], BF16, tag="xs", bufs=2)
                for t in range(2):
                    nc.vector.tensor_mul(xs_q[:, t, :], xbcc[:, t, sl], wbc_q[:, t, :])
                    eng = nc.sync if t == 0 else nc.scalar
                    eng.dma_start_transpose(
                        out=XT[:, 8 * q:8 * (q + 1), t * 128:(t + 1) * 128],
                        in_=xs_q[:, t, :])

            # ============ phase C: small tail preps, then scan ============
            with contextlib.ExitStack() as pctx:
                psC = pctx.enter_context(tc.tile_pool(name="psC", bufs=1, space="PSUM"))
                # ---- C at positions (conv window matmul + 4-tap conv + silu)
                C32 = sb.tile([128, NPOS], F32)
                pcw = psC.tile([128, NPOS * D_CONV], F32, tag="pcw")
                for k in range(KT):
                    nc.tensor.matmul(pcw, w_c_sb[:, k, :], xtw_sb[:, k, :],
                                     start=(k == 0), stop=(k == KT - 1))
                tmpc = sb.tile([128, NPOS], F32)
                for j in range(D_CONV):
                    src = bass.AP(tensor=pcw.tensor, offset=pcw.offset + j,
                                  ap=[list(pcw.ap[0]), [D_CONV, NPOS]])
                    if j == 0:
                        nc.vector.tensor_scalar_mul(tmpc, src, cw_sb[:, 0:1])
                    else:
                        nc.vector.scalar_tensor_tensor(
                            out=tmpc, in0=src, scalar=cw_sb[:, j:j + 1], in1=tmpc,
                            op0=OP.mult, op1=OP.add)
                nc.scalar.activation(out=C32, in_=tmpc, func=AF.Silu,
                                     bias=cbc_sb[:, 0:1], scale=1.0)
                C32b = sb.tile([128, NPOS], BF16)
                nc.vector.tensor_copy(out=C32b, in_=C32)

                # ---- z gate values at positions
                pz = psC.tile([128, 2, NPOS], F32, tag="pz")
                for t in range(2):
                    for k in range(KT):
                        nc.tensor.matmul(pz[:, t, :],
                                         w_z_sb[:, k, t * 128:(t + 1) * 128],
                                         xtp_sb[:, k, :],
                                         start=(k == 0), stop=(k == KT - 1))
                zs = sb.tile([128, 2, NPOS], F32)
                nc.scalar.activation(out=zs, in_=pz, func=AF.Silu)

                # ---- BC row = sum_n B32*C32 -> [1, NPOS] -> (h,c) layout
                bc_t = sb.tile([128, NPOS], F32)
                nc.vector.tensor_mul(bc_t, B32, C32)
                ones1 = sb.tile([128, 1], F32)
                nc.vector.memset(ones1, 1.0)
                pbc = psC.tile([1, NPOS], F32, tag="pbc")
                nc.tensor.matmul(pbc, ones1, bc_t, start=True, stop=True)
                bc_row = sb.tile([1, NPOS], F32)
                nc.vector.tensor_copy(out=bc_row, in_=pbc)
                bc_d = dsc.tile([1, NPOS], F32)
                nc.gpsimd.dma_start(out=bc_d, in_=bc_row)
                bc_hc = sb.tile([128, 1], F32)
                nc.gpsimd.dma_start(
                    out=bc_hc,
                    in_=bass.AP(tensor=bc_d.tensor, offset=bc_d.offset,
                                ap=[[0, HPC], [1, NPOS], [0, 1]]))
                # f_hc = dt*.BC* + D  (per (h,c)=(h,pos) partition)
                f_hc = sb.tile([128, 1], F32)
                nc.vector.scalar_tensor_tensor(out=f_hc, in0=dt32[:, 0:1],
                                               scalar=bc_hc[:, 0:1], in1=D_sb,
                                               op0=OP.mult, op1=OP.add)
                # broadcast dA_hc, f_hc -> [128,(t,pos)] via DRAM
                small_d = dsc.tile([128, 2], F32)
                nc.gpsimd.dma_start(out=small_d[:, 0:1], in_=dA_hc)
                nc.gpsimd.dma_start(out=small_d[:, 1:2], in_=f_hc)

                def bcast_hp(col):
                    t_sb = sb.tile([128, 2, NPOS], F32, tag=f"bch{col}")
                    for t in range(2):
                        for hh in range(2):
                            src = bass.AP(
                                tensor=small_d.tensor,
                                offset=small_d.offset + col + (2 * t + hh) * 2 * NCHUNK,
                                ap=[[0, 64], [2, NPOS]])
                            nc.gpsimd.dma_start(
                                out=t_sb[64 * hh:64 * (hh + 1), t, :], in_=src)
                    return t_sb

                dA_bc = bcast_hp(0)
                f_bc = bcast_hp(1)

                # ---- chunk-local states (recycles the xbcc slot)
                S_all = sb.tile([128, HPC * HEADDIM * NCHUNK], F32, tag="big1")
                for c in range(NCHUNK):
                    # all 4 heads' columns are contiguous in XT -> one matmul
                    pc = psC.tile([128, HPC * HEADDIM], F32, tag="psc", bufs=4)
                    for k2 in range(2):
                        T = 2 * c + k2
                        nc.tensor.matmul(pc, BT[:, T, :], XT[:, T, :],
                                         start=(k2 == 0), stop=(k2 == 1))
                    dst = bass.AP(tensor=S_all.tensor, offset=S_all.offset + c,
                                  ap=[list(S_all.ap[0]),
                                      [HEADDIM * NCHUNK, HPC], [NCHUNK, HEADDIM]])
                    if c % 2 == 0:
                        nc.vector.tensor_copy(out=dst, in_=pc.rearrange(
                            "n (h p) -> n h p", h=HPC))
                    else:
                        nc.scalar.copy(out=dst, in_=pc.rearrange(
                            "n (h p) -> n h p", h=HPC))
                # inter-chunk recurrence: state = lam*state + S_local along c
                # (Sg recycles the XT slot -- all scan matmuls have read XT by now)
                Sg = sb.tile([128, HPC * HEADDIM * NCHUNK], BF16, tag="big2")
                nc.vector.tensor_tensor_scan(out=Sg, data0=lam_flat, data1=S_all,
                                             initial=0.0, op0=OP.mult, op1=OP.add)

                # per-position projections: y32s[(hh,p), t, pos] = C32 . Sg[:,(h,p,pos-1)]
                py = psC.tile([128, 2, NPOS], F32, tag="py")
                nc.vector.memset(py[:, :, 0:1], 0.0)
                first = True
                for pos in range(1, NPOS):
                    for t in range(2):
                        lhs = bass.AP(
                            tensor=Sg.tensor,
                            offset=Sg.offset + (2 * t) * HEADDIM * NCHUNK + (pos - 1),
                            ap=[list(Sg.ap[0]), [HEADDIM * NCHUNK, 2], [NCHUNK, 64]])
                        nc.tensor.matmul(py[:, t, pos:pos + 1], lhs,
                                         C32b[:, pos:pos + 1],
                                         start=first, stop=(pos == NPOS - 1 and t == 1),
                                         skip_group_check=True)
                        first = False
                y32 = sb.tile([128, 2, NPOS], F32)
                nc.vector.tensor_mul(y32, py, dA_bc)
                tloc = sb.tile([128, 2, NPOS], F32)
                nc.vector.tensor_mul(tloc, x32, f_bc)
                nc.vector.tensor_add(y32, y32, tloc)
                nc.vector.tensor_mul(y32, y32, zs)
                nc.sync.dma_start(out=y32g_out, in_=y32)

    nc.compile()
    _L1 = nc
    return nc


# ----------------------------------------------------------------------------
# Launch 2 program: gated RMSNorm + out_proj + MLP on the 32 rows
# ----------------------------------------------------------------------------
_L2 = None


def build_l2():
    global _L2
    if _L2 is not None:
        return _L2
    nc = bacc.Bacc("TRN2", target_bir_lowering=False, debug=False,
                   num_devices=NCORES)

    y32g = nc.dram_tensor("y32g_full", (128, 16, NPOS), F32, kind="ExternalInput").ap()
    norm_w = nc.dram_tensor("norm_w", (128, 16), F32, kind="ExternalInput").ap()
    w_out = nc.dram_tensor("w_outT", (16, 128, D_MODEL), BF16, kind="ExternalInput").ap()
    w1 = nc.dram_tensor("w1T", (KT, 128, HIDDEN), BF16, kind="ExternalInput").ap()
    b1 = nc.dram_tensor("b1", (128, HIDDEN // 128), F32, kind="ExternalInput").ap()
    w2 = nc.dram_tensor("w2T", (32, 128, 512), BF16, kind="ExternalInput").ap()
    b2 = nc.dram_tensor("b2", (128, 4), F32, kind="ExternalInput").ap()
    out32 = nc.dram_tensor("out32", (128, 4, NPOS), F32, kind="ExternalOutput").ap()

    with tile.TileContext(nc) as tc:
        import contextlib
        with contextlib.ExitStack() as ctx:
            sb = ctx.enter_context(tc.tile_pool(name="sb", bufs=1))
            psp = ctx.enter_context(tc.tile_pool(name="ps", bufs=1, space="PSUM"))
            dsc = ctx.enter_context(tc.tile_pool(name="dsc", bufs=1, space="DRAM"))

            y_sb = sb.tile([128, 16, NPOS], F32)
            nc.sync.dma_start(out=y_sb, in_=y32g)
            nw_sb = sb.tile([128, 16], F32)
            nc.sync.dma_start(out=nw_sb, in_=norm_w)
            wo_sb = sb.tile([128, 16, D_MODEL], BF16)
            nc.sync.dma_start(out=wo_sb, in_=w_out.rearrange("k p c -> p k c"))
            w1_sb = sb.tile([128, KT, HIDDEN], BF16)
            nc.sync.dma_start(out=w1_sb, in_=w1.rearrange("k p c -> p k c"))
            b1_sb = sb.tile([128, HIDDEN // 128], F32)
            nc.sync.dma_start(out=b1_sb, in_=b1)
            w2_sb = sb.tile([128, 32, 512], BF16)
            nc.sync.dma_start(out=w2_sb, in_=w2.rearrange("k p c -> p k c"))
            b2_sb = sb.tile([128, 4], F32)
            nc.sync.dma_start(out=b2_sb, in_=b2)

            # sum of squares over channels (partition x 16 ktiles)
            sq = sb.tile([128, 16, NPOS], F32)
            nc.vector.tensor_mul(sq, y_sb, y_sb)
            ones1 = sb.tile([128, 1], F32)
            nc.vector.memset(ones1, 1.0)
            pss = psp.tile([1, NPOS], F32, tag="pss")
            for k in range(16):
                nc.tensor.matmul(pss, ones1, sq[:, k, :],
                                 start=(k == 0), stop=(k == 15))
            # r = 1/sqrt(mean + eps)
            eps_t = sb.tile([1, 1], F32)
            nc.vector.memset(eps_t, 1e-5)
            rs = sb.tile([1, NPOS], F32)
            nc.scalar.activation(out=rs, in_=pss, func=AF.Sqrt,
                                 bias=eps_t[:, 0:1], scale=1.0 / D_INNER)
            nc.vector.reciprocal(rs, rs)
            r_d = dsc.tile([1, NPOS], F32)
            nc.sync.dma_start(out=r_d, in_=rs)
            r_bc = sb.tile([128, NPOS], F32)
            nc.sync.dma_start(out=r_bc,
                              in_=bass.AP(tensor=r_d.tensor, offset=r_d.offset,
                                          ap=[[0, 128], [1, NPOS]]))
            yn = sb.tile([128, 16, NPOS], BF16)
            for k in range(16):
                nc.vector.scalar_tensor_tensor(out=yn[:, k, :], in0=y_sb[:, k, :],
                                               scalar=nw_sb[:, k:k + 1], in1=r_bc,
                                               op0=OP.mult, op1=OP.mult)
            # h = w_outT.T @ yn   [1024, 32]
            h_sb = sb.tile([128, 8, NPOS], BF16)
            for mt in range(8):
                ph = psp.tile([128, NPOS], F32, tag="ph", bufs=2)
                for k in range(16):
                    nc.tensor.matmul(ph, wo_sb[:, k, mt * 128:(mt + 1) * 128],
                                     yn[:, k, :], start=(k == 0), stop=(k == 15))
                nc.vector.tensor_copy(out=h_sb[:, mt, :], in_=ph)
            # g = gelu(w1T.T @ h + b1)  [4096, 32]
            g_sb = sb.tile([128, 32, NPOS], BF16)
            for mt in range(32):
                pg = psp.tile([128, NPOS], F32, tag="pg", bufs=2)
                for k in range(KT):
                    nc.tensor.matmul(pg, w1_sb[:, k, mt * 128:(mt + 1) * 128],
                                     h_sb[:, k, :], start=(k == 0), stop=(k == KT - 1))
                nc.scalar.activation(out=g_sb[:, mt, :], in_=pg, func=AF.Gelu,
                                     bias=b1_sb[:, mt:mt + 1], scale=1.0)
            # out = w2T.T @ g + b2   [512, 32] per core
            for mt in range(4):
                po = psp.tile([128, NPOS], F32, tag="po", bufs=2)
                for k in range(32):
                    nc.tensor.matmul(po, w2_sb[:, k, mt * 128:(mt + 1) * 128],
                                     g_sb[:, k, :], start=(k == 0), stop=(k == 31))
                ot = sb.tile([128, NPOS], F32, tag="ot", bufs=2)
                nc.vector.tensor_scalar(ot, po, b2_sb[:, mt:mt + 1], None, OP.add)
                nc.sync.dma_start(out=out32[:, mt, :], in_=ot)

    nc.compile()
    _L2 = nc
    return nc


# ----------------------------------------------------------------------------
# Host-side prep + glue
# ----------------------------------------------------------------------------

def _prep_l1_maps(inputs):
    x = _f32(inputs["x"]).reshape(L, D_MODEL)
    xT = np.ascontiguousarray(x.T)                       # [1024, 8192]
    xT_b = _bf(xT)
    pos = np.arange(NPOS) * POS_STRIDE
    xTpos = _bf(xT[:, pos])
    # window tokens (pos, d): t*-3+d, zero-padded below 0
    win_idx = (pos[:, None] + np.arange(D_CONV)[None, :] - (D_CONV - 1)).reshape(-1)
    xTwin = np.zeros((D_MODEL, NPOS * D_CONV), np.float32)
    valid = win_idx >= 0
    xTwin[:, valid] = xT[:, win_idx[valid]]
    xTwin = _bf(xTwin)

    w_all = _f32(inputs["in_proj_w"])                    # [4384, 1024]
    conv_w = _f32(inputs["conv_w"])                      # [2304, 4]
    conv_b = _f32(inputs["conv_b"])                      # [2304]
    dt_bias = _f32(inputs["dt_bias"])                    # [32]
    A = -np.exp(_f32(inputs["A_log"]))                   # [32]
    Dp = _f32(inputs["D"])                               # [32]

    w_cT = _bf(w_all[D_INNER + D_INNER + D_STATE:
                     D_INNER + D_INNER + 2 * D_STATE].T.reshape(KT, 128, 128))
    cw_c = _f32(conv_w[D_INNER + D_STATE:])              # [128, 4] C channels
    conv_b_c = _f32(conv_b[D_INNER + D_STATE:]).reshape(128, 1)

    maps = []
    for k in range(NCORES):
        xs = 256 * k
        cols = np.concatenate([
            np.arange(D_INNER + xs, D_INNER + xs + 256),          # x slice
            np.arange(2 * D_INNER, 2 * D_INNER + D_STATE),        # B
            np.arange(D_IN_PROJ - NHEADS + HPC * k,
                      D_IN_PROJ - NHEADS + HPC * k + HPC),        # dt
        ])
        w_in = _bf(w_all[cols].T.reshape(KT, 128, NCOL))
        w_z = _bf(w_all[xs:xs + 256].T.reshape(KT, 128, 256))
        # conv channels for this core: x slice (256) + B (128)
        ch_x = np.arange(xs, xs + 256)
        ch_B = np.arange(D_INNER, D_INNER + D_STATE)
        dw = np.zeros((3, D_CONV, 128, 128), np.float32)
        cb = np.zeros((128, 3), np.float32)
        for cht, chs in enumerate([ch_x[:128], ch_x[128:], ch_B]):
            for j in range(D_CONV):
                dw[cht, j] = np.diag(conv_w[chs, j])
            cb[:, cht] = conv_b[chs]
        heads = np.arange(HPC * k, HPC * k + HPC)
        dtb_hc = np.repeat(dt_bias[heads], NCHUNK).reshape(128, 1).astype(np.float32)
        A_hc = np.repeat(A[heads], NCHUNK).reshape(128, 1).astype(np.float32)
        D_hc = np.repeat(Dp[heads], NCHUNK).reshape(128, 1).astype(np.float32)
        mask = np.ones((128, 1), np.float32)
        mask[::NCHUNK] = 0.0
        maps.append({
            "xT": xT_b, "xTpos": xTpos, "xTwin": xTwin,
            "w_in": w_in, "w_c": w_cT, "w_z": w_z,
            "diag_w": _bf(dw), "cw_c": cw_c, "conv_b": cb,
            "conv_b_c": conv_b_c, "dtb_hc": dtb_hc, "A_hc": A_hc,
            "D_hc": D_hc, "mask_c0": mask,
        })
    return maps


def _prep_l2_maps(inputs, y32g_full):
    # ch = kt*128 + p -> norm_w_sb[p, kt] = norm_w[kt*128+p]
    nw = _f32(inputs["norm_w"]).reshape(16, 128).transpose(1, 0).copy()
    w_out = _f32(inputs["mamba_out_w"])                  # [1024, 2048]
    w_outT = _bf(w_out.T.reshape(16, 128, D_MODEL))
    w1 = _f32(inputs["mlp_w1"])                          # [4096, 1024]
    w1T = _bf(w1.T.reshape(KT, 128, HIDDEN))
    b1 = _f32(inputs["mlp_b1"]).reshape(32, 128).transpose(1, 0).copy()
    w2 = _f32(inputs["mlp_w2"])                          # [4096, 4096]
    maps = []
    for k in range(NCORES):
        cols = slice(512 * k, 512 * k + 512)
        w2T = _bf(w2[cols].T.reshape(32, 128, 512))
        b2 = _f32(inputs["mlp_b2"])[cols].reshape(4, 128).transpose(1, 0).copy()
        maps.append({
            "y32g_full": y32g_full, "norm_w": nw, "w_outT": w_outT,
            "w1T": w1T, "b1": b1, "w2T": w2T, "b2": b2,
        })
    return maps


LAST_RESULTS = []


def kernel(**inputs) -> np.ndarray:
    trace = os.environ.get("KERNEL_TRACE", "0") == "1"
    LAST_RESULTS.clear()
    nc1 = build_l1()
    maps1 = _prep_l1_maps(inputs)
    res1 = bass_utils.run_bass_kernel_spmd(nc1, maps1, core_ids=list(range(NCORES)),
                                           trace=trace)
    LAST_RESULTS.append(res1)
    # assemble y32g_full [128, 16, 32]: ch = 256*k + t*128 + p -> kt = 2k+t
    y32g_full = np.zeros((128, 16, NPOS), np.float32)
    for k in range(NCORES):
        y = res1.results[k]["y32g"]                      # [128, 2, 32]
        y32g_full[:, 2 * k:2 * k + 2, :] = y
    nc2 = build_l2()
    maps2 = _prep_l2_maps(inputs, y32g_full)
    res2 = bass_utils.run_bass_kernel_spmd(nc2, maps2, core_ids=list(range(NCORES)),
                                           trace=trace)
    LAST_RESULTS.append(res2)
    out = np.zeros((NPOS, HIDDEN), np.float32)
    for k in range(NCORES):
        o = res2.results[k]["out32"]                     # [128, 4, 32]
        # out[pos, 512k + mt*128 + p] = o[p, mt, pos]
        out[:, 512 * k:512 * (k + 1)] = o.transpose(2, 1, 0).reshape(NPOS, 512)
    return out.astype(np.float32)
